# revision 1
# baseline (speedup 1.0000x reference)
"""Trainium2 Bass kernel for nn_ConditionalPreactivation.

Reference computation (B=4096, DIN=DOUT=512, DC=64, K=16, DB=256):
    a  = lrelu(LayerNorm(x) * gamma + beta)            [B, DIN]
    h  = c @ w1 + b1; h = h + lrelu(h) @ wr + br; h = lrelu(h)
    bf = h @ w2 + b2                                   [B, K]
    out[b, o] = sum_k bf[b,k] * (a[b] @ W[k])[o] + (bf @ bvec)[b, o]

Strategy: data-parallel over batch across 8 cores (512 rows each); W
(16MB) replicated and streamed from DRAM.  Everything on-device runs in
"transposed" layout (features on partitions, batch on the free dim) so
no on-device transposes of activations are needed; x and c are
transposed on the host as sharding prep.  The dominant matmul
(2*B*K*DIN*DOUT = 34 GFLOP) runs in fp32r (bf16 hi+lo pair format,
1 cycle/row on the PE at N=512, ~1e-4 relative error).  Per k, the
partial y_k = a @ W_k accumulates in PSUM over 4 contraction tiles and
is drained with a fused DVE op: out_acc += bf[:, k] * y_k.
"""

import numpy as np
import ml_dtypes

import concourse.bacc as bacc
import concourse.bass as bass
import concourse.mybir as mybir
import concourse.tile as tile
from concourse.bass_utils import run_bass_kernel_spmd
from concourse.masks import make_identity

F32 = mybir.dt.float32
F32R = mybir.dt.float32r
ALU = mybir.AluOpType
ACTF = mybir.ActivationFunctionType

B, DIN, DOUT, DC, K, DB = 4096, 512, 512, 64, 16, 256
NEG = 0.01
LN_EPS = 1e-5
NCORES = 8
BS = B // NCORES          # 512 batch rows per core
NIT = DIN // 128          # 4 contraction tiles of the a @ W_k matmul
NBT = BS // 128           # 4 batch tiles (output partition tiles)


def _round_fp32r(x):
    """Round fp32 to the bf16 hi+lo pair grid the fp32r matmul uses."""
    x = np.ascontiguousarray(x, dtype=np.float32)
    hi = x.astype(ml_dtypes.bfloat16).astype(np.float32)
    lo = (x - hi).astype(ml_dtypes.bfloat16).astype(np.float32)
    return hi + lo


def build_nc():
    nc = bacc.Bacc("TRN2", target_bir_lowering=False)

    # --- per-core inputs (host-prepped layouts) ---
    # xTt[p, t, b] = x_shard.T[t*128 + p, b]
    xTt = nc.dram_tensor("xTt", [128, NIT, BS], F32R, kind="ExternalInput")
    cT = nc.dram_tensor("cT", [DC, BS], F32R, kind="ExternalInput")
    w1 = nc.dram_tensor("w1", [DC, DB], F32R, kind="ExternalInput")
    # wr as [128, 2, 256]: wrT[p, t, m] = wr[t*128 + p, m]
    wrT = nc.dram_tensor("wrT", [128, 2, DB], F32R, kind="ExternalInput")
    # w2 as [128, 2, 16]
    w2T = nc.dram_tensor("w2T", [128, 2, K], F32R, kind="ExternalInput")
    b1T = nc.dram_tensor("b1T", [128, 2], F32, kind="ExternalInput")
    brT = nc.dram_tensor("brT", [128, 2], F32, kind="ExternalInput")
    b2c = nc.dram_tensor("b2c", [K, 1], F32, kind="ExternalInput")
    gammaT = nc.dram_tensor("gammaT", [128, NIT], F32, kind="ExternalInput")
    betaT = nc.dram_tensor("betaT", [128, NIT], F32, kind="ExternalInput")
    onc = nc.dram_tensor("onc", [128, 1], F32R, kind="ExternalInput")
    onr = nc.dram_tensor("onr", [1, 128], F32R, kind="ExternalInput")
    onc32 = nc.dram_tensor("onc32", [128, 1], F32, kind="ExternalInput")
    id128 = nc.dram_tensor("id128", [128, 128], F32R, kind="ExternalInput")
    # W bank: wbank[p, ct, o] = W.reshape(K*DIN, DOUT)[ct*128 + p, o]
    wbank = nc.dram_tensor("wbank", [128, K * NIT, DOUT], F32R,
                           kind="ExternalInput")
    bvec = nc.dram_tensor("bvec", [K, DOUT], F32R, kind="ExternalInput")
    out = nc.dram_tensor("out", [BS, DOUT], F32, kind="ExternalOutput")

    with tile.TileContext(nc) as tc, \
         tc.tile_pool(name="persist", bufs=1) as pp, \
         tc.tile_pool(name="wpool", bufs=10) as wp:

        # ---- resident tensors (small DMAs first: they gate the first
        # stats matmuls and share the ACT HWDGE ring FIFO) ----
        ones_col = pp.tile([128, 1], F32R, name="ones_col")
        nc.scalar.dma_start(out=ones_col, in_=onc[:, :])
        ones_row = pp.tile([1, 128], F32R, name="ones_row")
        nc.scalar.dma_start(out=ones_row, in_=onr[:, :])
        ones32 = pp.tile([128, 1], F32, name="ones32")
        nc.scalar.dma_start(out=ones32, in_=onc32[:, :])
        id_sb = pp.tile([128, 128], F32R, name="id_sb")
        nc.scalar.dma_start(out=id_sb, in_=id128[:, :])
        cT_sb = pp.tile([DC, BS], F32R, name="cT_sb")
        nc.scalar.dma_start(out=cT_sb, in_=cT[:, :])
        w1_sb = pp.tile([DC, DB], F32R, name="w1_sb")
        nc.scalar.dma_start(out=w1_sb, in_=w1[:, :])
        wrT_sb = pp.tile([128, 2, DB], F32R, name="wrT_sb")
        nc.scalar.dma_start(out=wrT_sb, in_=wrT[:, :, :])
        w2T_sb = pp.tile([128, 2, K], F32R, name="w2T_sb")
        nc.scalar.dma_start(out=w2T_sb, in_=w2T[:, :, :])
        b1T_sb = pp.tile([128, 2], F32, name="b1T_sb")
        nc.scalar.dma_start(out=b1T_sb, in_=b1T[:, :])
        brT_sb = pp.tile([128, 2], F32, name="brT_sb")
        nc.scalar.dma_start(out=brT_sb, in_=brT[:, :])
        b2_sb = pp.tile([K, 1], F32, name="b2_sb")
        nc.scalar.dma_start(out=b2_sb, in_=b2c[:, :])
        gammaT_sb = pp.tile([128, NIT], F32, name="gammaT_sb")
        nc.scalar.dma_start(out=gammaT_sb, in_=gammaT[:, :])
        betaT_sb = pp.tile([128, NIT], F32, name="betaT_sb")
        nc.scalar.dma_start(out=betaT_sb, in_=betaT[:, :])
        bvec_sb = pp.tile([K, DOUT], F32R, name="bvec_sb")
        nc.scalar.dma_start(out=bvec_sb, in_=bvec[:, :])
        xT_sb = pp.tile([128, NIT, BS], F32R, name="xT_sb")
        for it in range(NIT):
            nc.sync.dma_start(out=xT_sb[:, it, :], in_=xTt[:, it, :])

        ident = pp.tile([K, K], F32, name="ident")
        make_identity(nc, ident)

        aT_sb = pp.tile([128, NIT, BS], F32R, name="aT_sb")
        bfT_sb = pp.tile([K, BS], F32, name="bfT_sb")
        bfT_r = pp.tile([K, BS], F32R, name="bfT_r")
        bfn_sb = pp.tile([128, NBT, K], F32, name="bfn_sb")
        out_acc = pp.tile([128, NBT, DOUT], F32, name="out_acc")

        with tc.tile_pool(name="phase1", bufs=1) as p1, \
             tc.tile_pool(name="psumA", bufs=3, space="PSUM") as psA:

            # ======== LayerNorm stats (partition reduction via matmul) ====
            xsq = p1.tile([128, NIT, BS], F32, name="xsq")
            for it in range(NIT):
                nc.gpsimd.tensor_mul(xsq[:, it, :],
                                     xT_sb[:, it, :].bitcast(F32),
                                     xT_sb[:, it, :].bitcast(F32))
            sum_ps = psA.tile([1, BS], F32, name="sum_ps", tag="ps")
            sumsq_ps = psA.tile([1, BS], F32, name="sumsq_ps", tag="ps")
            for it in range(NIT):
                nc.tensor.matmul(sum_ps, ones_col, xT_sb[:, it, :],
                                 start=(it == 0), stop=(it == NIT - 1))
            for it in range(NIT):
                nc.tensor.matmul(sumsq_ps, ones32, xsq[:, it, :],
                                 start=(it == 0), stop=(it == NIT - 1))

            mu = p1.tile([1, BS], F32, name="mu")
            nc.vector.tensor_scalar_mul(mu, sum_ps, 1.0 / DIN)
            mu_r = p1.tile([1, BS], F32R, name="mu_r")
            nc.vector.tensor_copy(mu_r, mu)
            nmu_r = p1.tile([1, BS], F32R, name="nmu_r")
            nc.vector.tensor_scalar_mul(nmu_r, sum_ps, -1.0 / DIN)
            musq = p1.tile([1, BS], F32, name="musq")
            nc.vector.tensor_mul(musq, mu, mu)
            var = p1.tile([1, BS], F32, name="var")
            # var = sumsq/DIN - mu^2
            nc.vector.scalar_tensor_tensor(var, sumsq_ps, 1.0 / DIN, musq,
                                           op0=ALU.mult, op1=ALU.subtract)
            eps_t = p1.tile([1, 1], F32, name="eps_t")
            nc.vector.memset(eps_t, LN_EPS)
            sd = p1.tile([1, BS], F32, name="sd")
            nc.scalar.activation(sd, var, ACTF.Sqrt, bias=eps_t[:, 0:1],
                                 scale=1.0)
            rstd = p1.tile([1, BS], F32, name="rstd")
            nc.vector.reciprocal(rstd, sd)
            rstd_r = p1.tile([1, BS], F32R, name="rstd_r")
            nc.vector.tensor_copy(rstd_r, rstd)

            # broadcast mu / rstd across partitions via ones-matmul
            rs_bp = psA.tile([128, BS], F32, name="rs_bp", tag="ps")
            nc.tensor.matmul(rs_bp, ones_row, rstd_r, start=True, stop=True)
            # drain to SBUF once; read by 4 DVE muls below
            rs_b = p1.tile([128, BS], F32, name="rs_b")
            nc.scalar.activation(rs_b, rs_bp, ACTF.Copy, bias=0.0, scale=1.0)

            # aT = lrelu(gamma * (xT - mu) * rstd + beta), lrelu(v)=max(v, .01v)
            for it in range(NIT):
                # cen = xT - bcast(mu), built on the PE: id @ xT - ones @ mu
                cen_ps = psA.tile([128, BS], F32, name="cen_ps",
                                  tag="cen_ps", bufs=2)
                nc.tensor.matmul(cen_ps, id_sb, xT_sb[:, it, :],
                                 start=True, stop=False)
                nc.tensor.matmul(cen_ps, ones_row, nmu_r,
                                 start=False, stop=True)
                nrm = p1.tile([128, BS], F32, name="nrm", tag="nrm", bufs=2)
                nc.vector.tensor_mul(nrm, cen_ps, rs_b)
                pre = p1.tile([128, BS], F32, name="pre", tag="pre", bufs=2)
                nc.scalar.activation(pre, nrm, ACTF.Identity,
                                     bias=betaT_sb[:, it:it + 1],
                                     scale=gammaT_sb[:, it:it + 1])
                nc.vector.scalar_tensor_tensor(aT_sb[:, it, :], pre, NEG, pre,
                                               op0=ALU.mult, op1=ALU.max)

            # ======== basis functions MLP (transposed layout) ========
            h1_ps = psA.tile([128, 2, BS], F32, name="h1_ps", tag="ps2",
                             bufs=1)
            for mt in range(2):
                nc.tensor.matmul(h1_ps[:, mt, :], w1_sb[:, bass.ts(mt, 128)],
                                 cT_sb, start=True, stop=True)
            h1_sb = p1.tile([128, 2, BS], F32, name="h1_sb")
            g_f = p1.tile([128, 2, BS], F32, name="g_f")
            g_sb = p1.tile([128, 2, BS], F32R, name="g_sb")
            for mt in range(2):
                nc.scalar.activation(h1_sb[:, mt, :], h1_ps[:, mt, :],
                                     ACTF.Identity,
                                     bias=b1T_sb[:, mt:mt + 1], scale=1.0)
                nc.scalar.activation(g_f[:, mt, :], h1_ps[:, mt, :],
                                     ACTF.Prelu,
                                     bias=b1T_sb[:, mt:mt + 1], scale=1.0,
                                     alpha=NEG)
                nc.vector.tensor_copy(g_sb[:, mt, :], g_f[:, mt, :])
            rT_ps = psA.tile([128, 2, BS], F32, name="rT_ps", tag="ps2",
                             bufs=1)
            for mt in range(2):
                for t in range(2):
                    nc.tensor.matmul(rT_ps[:, mt, :],
                                     wrT_sb[:, t, bass.ts(mt, 128)],
                                     g_sb[:, t, :],
                                     start=(t == 0), stop=(t == 1))
            hact = p1.tile([128, 2, BS], F32R, name="hact")
            for mt in range(2):
                ht = p1.tile([128, BS], F32, name="ht", tag="ht", bufs=2)
                # h = h1 + (rT + br)
                nc.vector.scalar_tensor_tensor(ht, rT_ps[:, mt, :],
                                               brT_sb[:, mt:mt + 1],
                                               h1_sb[:, mt, :],
                                               op0=ALU.add, op1=ALU.add)
                nc.vector.scalar_tensor_tensor(hact[:, mt, :], ht, NEG, ht,
                                               op0=ALU.mult, op1=ALU.max)
            bf_ps = psA.tile([K, BS], F32, name="bf_ps", tag="ps")
            for t in range(2):
                nc.tensor.matmul(bf_ps, w2T_sb[:, t, :], hact[:, t, :],
                                 start=(t == 0), stop=(t == 1))
            nc.scalar.activation(bfT_sb, bf_ps, ACTF.Identity,
                                 bias=b2_sb[:, 0:1], scale=1.0)
            nc.vector.tensor_copy(bfT_r, bfT_sb)

            # bf back to normal layout [b, k] per batch tile (PE transpose)
            for bt in range(NBT):
                tp = psA.tile([128, K], F32, name="tp", tag="ps")
                nc.tensor.transpose(tp, bfT_sb[:, bass.ts(bt, 128)], ident)
                nc.scalar.activation(bfn_sb[:, bt, :], tp, ACTF.Copy,
                                     bias=0.0, scale=1.0)

            # out_acc init with the bvec term: out_acc[bt] = bf_n @ bvec
            for bt in range(NBT):
                yv = psA.tile([128, DOUT], F32, name="yv", tag="ps")
                nc.tensor.matmul(yv, bfT_r[:, bass.ts(bt, 128)], bvec_sb,
                                 start=True, stop=True)
                nc.scalar.activation(out_acc[:, bt, :], yv, ACTF.Copy,
                                     bias=0.0, scale=1.0)

        # ======== main loop: out_acc[bt] += bf[:,k] * (a @ W_k) ========
        # Drains alternate between DVE (fused stt) and ACT (scale-copy) +
        # DVE add so neither engine becomes the bottleneck next to PE.
        with tc.tile_pool(name="psumC", bufs=8, space="PSUM") as psC:
            for k in range(K):
                wk = wp.tile([128, NIT, DOUT], F32R, name="wk", tag="wk")
                nc.sync.dma_start(out=wk, in_=wbank[:, bass.ts(k, NIT), :])
                for bt in range(NBT):
                    yk = psC.tile([128, DOUT], F32, name="yk", tag="yk")
                    for it in range(NIT):
                        nc.tensor.matmul(yk,
                                         aT_sb[:, it, bass.ts(bt, 128)],
                                         wk[:, it, :],
                                         start=(it == 0), stop=(it == NIT - 1))
                    nc.vector.scalar_tensor_tensor(
                        out_acc[:, bt, :], yk, bfn_sb[:, bt, k:k + 1],
                        out_acc[:, bt, :], op0=ALU.mult, op1=ALU.add)

        for bt in range(NBT):
            nc.scalar.dma_start(out=out.ap()[bass.ts(bt, 128), :],
                                in_=out_acc[:, bt, :])

    nc.compile()
    return nc


_NC_CACHE = None


def _get_nc():
    global _NC_CACHE
    if _NC_CACHE is None:
        _NC_CACHE = build_nc()
    return _NC_CACHE


def kernel(x, c, ln_gamma, ln_beta, w1, b1, wr, br, w2, b2, W, bvec):
    x = np.asarray(x, dtype=np.float32)
    c = np.asarray(c, dtype=np.float32)

    # shared (replicated) host-prepped tensors
    w1_r = _round_fp32r(w1)
    wrT = _round_fp32r(np.asarray(wr, np.float32)
                       .reshape(2, 128, DB).transpose(1, 0, 2))
    w2T = _round_fp32r(np.asarray(w2, np.float32)
                       .reshape(2, 128, K).transpose(1, 0, 2))
    b1T = np.asarray(b1, np.float32).reshape(2, 128).T.copy()
    brT = np.asarray(br, np.float32).reshape(2, 128).T.copy()
    b2c = np.asarray(b2, np.float32).reshape(K, 1).copy()
    gammaT = np.asarray(ln_gamma, np.float32).reshape(NIT, 128).T.copy()
    betaT = np.asarray(ln_beta, np.float32).reshape(NIT, 128).T.copy()
    # wbank[p, ct, o] = Wf[ct*128 + p, o],  Wf = W.reshape(K*DIN, DOUT)
    wbank = _round_fp32r(np.asarray(W, np.float32)
                         .reshape(K * NIT, 128, DOUT).transpose(1, 0, 2))
    bvec_r = _round_fp32r(bvec)
    onc = np.ones((128, 1), np.float32)
    onr = np.ones((1, 128), np.float32)
    id128 = np.eye(128, dtype=np.float32)

    shared = dict(w1=w1_r, wrT=wrT, w2T=w2T, b1T=b1T, brT=brT, b2c=b2c,
                  gammaT=gammaT, betaT=betaT, wbank=wbank, bvec=bvec_r,
                  onc=onc, onr=onr, onc32=onc, id128=id128)

    in_maps = []
    for core in range(NCORES):
        sl = slice(core * BS, (core + 1) * BS)
        xt = _round_fp32r(x[sl].T)                      # [DIN, BS]
        in_maps.append(dict(
            xTt=np.ascontiguousarray(
                xt.reshape(NIT, 128, BS).transpose(1, 0, 2)),
            cT=_round_fp32r(c[sl].T),
            **shared,
        ))

    nc = _get_nc()
    res = run_bass_kernel_spmd(nc, in_maps, core_ids=list(range(NCORES)))
    return np.concatenate([r["out"] for r in res.results], axis=0)

